# revision 17
# baseline (speedup 1.0000x reference)
"""GroupingBlock Bass/Tile kernel for 8 Trainium2 NeuronCores.

Data-parallel over batch B=32 -> 4 batches/core, weights replicated.
Host preprocessing folds LN gammas/betas into adjacent matmul weights where
algebraically exact, pre-broadcasts free-dim biases, casts matmul operands
to bf16, and lays weights out as [128, K//128, N] kxm tiles.

On-chip per batch: standardize x (per-token mean/rstd) -> bf16 -> DRAM
roundtrip with an XBAR DMA-transpose to get x_std^T [768, 4096]; all
projections contract over channels with x_std^T as an operand. Attention
runs in transposed layout (logits^T [n, heads*g]): softmax normalizers come
from ones-matmuls (no max subtraction needed; logits are O(1)), the
unnormalized exp weights feed attn@v directly, and 1/sum is applied to the
accumulated output. The hard assignment is a free-dim reduce_max + is_ge
mask (exact one-hot up to measure-zero ties).
"""

import os
import sys
import numpy as np
import ml_dtypes

if '/opt/trn_rl_repo' not in sys.path:
    sys.path.insert(0, '/opt/trn_rl_repo')

B, N, G_IN, G_OUT, C, H = 32, 4096, 128, 64, 768, 12
TOK_HID, MLP_HID, CH_HID = 384, 3072, 3072
NCORES = 8
BL = B // NCORES          # 4 batches per core
P = 128
KO = C // P               # 6 contraction chunks over channels
HD = C // H               # 64 head dim
NCH = N // P              # 32 n-chunks per batch
NSL = 512                 # n-slice for kT production
NQ = 512                  # xnT slice size
F32 = np.float32
BF16 = ml_dtypes.bfloat16


def _r3(w):
    """[C_in, M] -> [128, C_in//128, M] kxm layout, bf16."""
    return np.ascontiguousarray(
        w.reshape(-1, P, w.shape[-1]).transpose(1, 0, 2).astype(BF16))


def _bias_part(b):
    """[D] -> [128, D//128] per-partition bias layout (f32)."""
    return np.ascontiguousarray(b.reshape(-1, P).T.astype(F32))


def _bc(b):
    """[D] -> [128, D] broadcast (bf16)."""
    return np.ascontiguousarray(
        np.broadcast_to(b.astype(F32), (P, len(b))).astype(BF16))


def prep_weights(inp):
    """Host-side preprocessing -> dict of staged arrays (shared by cores)."""
    g = lambda k: np.asarray(inp[k], F32)
    w = {}
    lxg, lxb = g('ln_x_g'), g('ln_x_b')
    for nm, wk, bk in (('k', 'ca_kw', 'ca_kb'), ('v', 'ca_vw', 'ca_vb'),
                       ('ak', 'as_kw', 'as_kb'), ('av', 'as_vw', 'as_vb')):
        W = g(wk) * lxg[:, None]
        be = g(bk) + lxb @ g(wk)
        w['w_' + nm] = _r3(W)
        if nm in ('k', 'ak'):
            w['b_' + nm] = _bias_part(be)
        else:
            w['bc_' + nm] = _bc(be)
    s1 = float(HD) ** -0.5
    w['w_q'] = _r3(g('ca_qw') * s1)
    w['b_q'] = _bias_part(g('ca_qb') * s1)
    s2 = float(C) ** -0.5
    w['w_asq'] = _r3(g('as_qw') * s2)
    w['b_asq'] = _bias_part(g('as_qb') * s2)
    w['w_p'] = _r3(g('ca_pw'))
    w['bc_p'] = _bc(g('ca_pb'))
    w['w_asp'] = _r3(g('as_pw'))
    w['bc_asp'] = _bc(g('as_pb'))
    w['w_i1'] = np.ascontiguousarray(g('inter_w1').astype(BF16))
    w['b_i1'] = _bias_part(g('inter_b1'))
    w['w_i2'] = _r3(g('inter_w2'))
    w['b_i2'] = np.ascontiguousarray(
        np.tile(g('inter_b2'), 2)[:, None].astype(F32))
    for nm, lng, lnb, w1k, b1k, w2k, b2k in (
            ('m', 'ca_ln2_g', 'ca_ln2_b', 'ca_m1w', 'ca_m1b', 'ca_m2w', 'ca_m2b'),
            ('c', 'ln_nx_g', 'ln_nx_b', 'mc_w1', 'mc_b1', 'mc_w2', 'mc_b2')):
        W1 = g(w1k) * g(lng)[:, None]
        b1 = g(b1k) + g(lnb) @ g(w1k)
        w['w_%s1' % nm] = _r3(W1)
        w['b_%s1' % nm] = _bias_part(b1)
        w['w_%s2' % nm] = _r3(g(w2k))
        w['bc_%s2' % nm] = _bc(g(b2k))
    for nm, gk, bk in (('tok', 'ln_tokens_g', 'ln_tokens_b'),
                       ('pt', 'ln_pt_g', 'ln_pt_b'),
                       ('lnp', 'ca_lnp_g', 'ca_lnp_b')):
        w['g_' + nm] = _bc(g(gk))
        w['bt_' + nm] = _bc(g(bk))
    return w


def build_nc():
    """Build the Bass module for one core."""
    import concourse.bass as bass
    import concourse.tile as tile
    import concourse.mybir as mybir
    from concourse.masks import make_identity
    from contextlib import ExitStack

    dt = mybir.dt
    AF = mybir.ActivationFunctionType
    OP = mybir.AluOpType
    AX = mybir.AxisListType

    nc = bass.Bass('TRN2', target_bir_lowering=False, debug=False,
                   num_devices=NCORES)

    x_l = nc.dram_tensor('x_l', (BL, N, C), dt.float32, kind='ExternalInput').ap()
    gt_l = nc.dram_tensor('gt_l', (BL, G_IN, C), dt.float32,
                          kind='ExternalInput').ap()
    wspec = [
        ('w_k', (P, KO, C), dt.bfloat16), ('w_v', (P, KO, C), dt.bfloat16),
        ('w_ak', (P, KO, C), dt.bfloat16), ('w_av', (P, KO, C), dt.bfloat16),
        ('b_k', (P, KO), dt.float32), ('b_ak', (P, KO), dt.float32),
        ('bc_v', (P, C), dt.bfloat16), ('bc_av', (P, C), dt.bfloat16),
        ('w_q', (P, KO, C), dt.bfloat16), ('b_q', (P, KO), dt.float32),
        ('w_asq', (P, KO, C), dt.bfloat16), ('b_asq', (P, KO), dt.float32),
        ('w_p', (P, KO, C), dt.bfloat16), ('bc_p', (P, C), dt.bfloat16),
        ('w_asp', (P, KO, C), dt.bfloat16), ('bc_asp', (P, C), dt.bfloat16),
        ('w_i1', (P, TOK_HID), dt.bfloat16), ('b_i1', (P, 3), dt.float32),
        ('w_i2', (P, 3, G_OUT), dt.bfloat16), ('b_i2', (P, 1), dt.float32),
        ('w_m1', (P, KO, MLP_HID), dt.bfloat16), ('b_m1', (P, 24), dt.float32),
        ('w_m2', (P, 24, C), dt.bfloat16), ('bc_m2', (P, C), dt.bfloat16),
        ('w_c1', (P, KO, CH_HID), dt.bfloat16), ('b_c1', (P, 24), dt.float32),
        ('w_c2', (P, 24, C), dt.bfloat16), ('bc_c2', (P, C), dt.bfloat16),
        ('g_tok', (P, C), dt.bfloat16), ('bt_tok', (P, C), dt.bfloat16),
        ('g_pt', (P, C), dt.bfloat16), ('bt_pt', (P, C), dt.bfloat16),
        ('g_lnp', (P, C), dt.bfloat16), ('bt_lnp', (P, C), dt.bfloat16),
    ]
    wdram = {nm: nc.dram_tensor(nm, shp, d, kind='ExternalInput').ap()
             for nm, shp, d in wspec}
    out_l = nc.dram_tensor('out_l', (BL, G_OUT, C), dt.float32,
                           kind='ExternalOutput').ap()

    RESIDENT = ['w_k', 'w_v', 'w_ak', 'w_av', 'b_k', 'b_ak', 'bc_v', 'bc_av',
                'b_q', 'b_asq', 'bc_p', 'bc_asp', 'w_i1', 'b_i1', 'w_i2',
                'b_i2', 'b_m1', 'bc_m2', 'b_c1', 'bc_c2',
                'g_tok', 'bt_tok', 'g_pt', 'bt_pt', 'g_lnp', 'bt_lnp']

    class _NP:
        """tile_pool wrapper that auto-names tiles (name inference fails
        when the tile is sliced on the allocation line)."""
        _i = [0]

        def __init__(self, pool):
            self._pool = pool

        def tile(self, shape, dtype, tag, name=None):
            self._i[0] += 1
            return self._pool.tile(shape, dtype, tag=tag,
                                   name=name or '%s_%d' % (tag, self._i[0]))

    def _install_wait_splitter(tc):
        # The neuronxcc walrus in this container accepts at most ONE sync
        # wait per instruction; Tile emits several. Split extras into
        # standalone EventSemaphore ops (same engine, immediately before),
        # and split the tail drain's waits across multiple drains.
        orig_add = tc._add_instruction
        seq = [0]

        def patched_add(inst):
            si = inst.sync_info
            if si is not None and len(si.on_wait) > 1:
                waits = list(si.on_wait)
                for w in waits[:-1]:
                    seq[0] += 1
                    es = mybir.InstEventSemaphore(
                        name='I-ws%d' % seq[0], ins=[], outs=[],
                        engine=inst.engine)
                    es.sync_info = mybir.SyncInfo(on_wait=[w], on_update=[])
                    orig_add(es)
                inst.sync_info = mybir.SyncInfo(on_wait=waits[-1:],
                                                on_update=list(si.on_update))
            orig_add(inst)

        tc._add_instruction = patched_add

        from concourse.vector_clock import ScopedClock

        def patched_dab(tick_clock, wait_clock):
            drain_bi = nc.sync.drain()
            wait_clock.add_sem_waits(
                drain_bi.ins, ScopedClock({None: tick_clock.global_clock}))
            mi = drain_bi.ins
            si = mi.sync_info
            if si is not None and len(si.on_wait) > 1:
                waits = list(si.on_wait)
                mi.sync_info = mybir.SyncInfo(on_wait=waits[:1],
                                              on_update=list(si.on_update))
                for w in waits[1:]:
                    d2 = nc.sync.drain()
                    d2.ins.sync_info = mybir.SyncInfo(on_wait=[w], on_update=[])
            nc.all_engine_barrier()
            assert tc.sems is not None
            popped = nc._tile_sem_poison_stack.pop()
            assert popped is tc._sem_poison
            nc.clear_and_free_semaphores(list(tc.sems.allocated().values()))
            nc.all_engine_barrier()

        tc._drain_and_barrier = patched_dab

    with tile.TileContext(nc) as tc, ExitStack() as ctx:
        _install_wait_splitter(tc)
        konst = ctx.enter_context(tc.tile_pool(name='konst', bufs=1))
        wstream = ctx.enter_context(tc.tile_pool(name='wstream', bufs=2))
        dram = ctx.enter_context(tc.tile_pool(name='dram', bufs=1, space='DRAM'))
        xin = ctx.enter_context(tc.tile_pool(name='xin', bufs=2))
        xbfp = ctx.enter_context(tc.tile_pool(name='xbfp', bufs=2))
        stat = ctx.enter_context(tc.tile_pool(name='stat', bufs=4))
        row = ctx.enter_context(tc.tile_pool(name='row', bufs=2))
        xntp = ctx.enter_context(tc.tile_pool(name='xntp', bufs=2))
        ktp = ctx.enter_context(tc.tile_pool(name='ktp', bufs=2))
        chk = ctx.enter_context(tc.tile_pool(name='chk', bufs=2))
        acc = ctx.enter_context(tc.tile_pool(name='acc', bufs=1))
        pairp = ctx.enter_context(tc.tile_pool(name='pairp', bufs=1))
        psA = ctx.enter_context(tc.tile_pool(name='psA', bufs=3, space='PSUM'))
        psB = ctx.enter_context(tc.tile_pool(name='psB', bufs=1, space='PSUM'))
        psS = ctx.enter_context(tc.tile_pool(name='psS', bufs=2, space='PSUM'))
        konst, wstream, dram, row = _NP(konst), _NP(wstream), _NP(dram), _NP(row)
        xin, xbfp, stat, xntp = _NP(xin), _NP(xbfp), _NP(stat), _NP(xntp)
        ktp, chk, acc, pairp = _NP(ktp), _NP(chk), _NP(acc), _NP(pairp)
        psA, psB, psS = _NP(psA), _NP(psB), _NP(psS)

        W = {}
        for nm in RESIDENT:
            t = konst.tile(list(wdram[nm].shape), wdram[nm].dtype, tag='k_' + nm)
            nc.sync.dma_start(t[:], wdram[nm][:])
            W[nm] = t
        ident_b = konst.tile([P, P], dt.bfloat16, tag='ident_b')
        make_identity(nc, ident_b[:])
        ident_f = konst.tile([P, P], dt.float32, tag='ident_f')
        make_identity(nc, ident_f[:])
        ones1 = konst.tile([P, 1], dt.bfloat16, tag='ones1')
        nc.vector.memset(ones1[:], 1.0)
        onesq = konst.tile([P, P], dt.bfloat16, tag='onesq')
        nc.vector.memset(onesq[:], 1.0)
        eps_t = konst.tile([P, 1], dt.float32, tag='eps_t')
        nc.vector.memset(eps_t[:], 1e-5)
        onesf = konst.tile([P, HD], dt.float32, tag='onesf')
        nc.vector.memset(onesf[:], 1.0)

        def t1(shape, dtype=dt.float32):
            return psA.tile(shape, dtype, tag='T1')

        def t2():
            return psB.tile([P, C], dt.float32, tag='T2')

        def std_apply(src, D, out, gam=None, bet=None, scr=None):
            """out = standardize(src) [* gam + bet]; src [Pp, D] f32."""
            Pp = src.shape[0]
            s = stat.tile([P, 1], dt.float32, tag='s')[:Pp]
            nc.vector.reduce_sum(s[:], src[:], axis=AX.X)
            sq = xbfp.tile([P, C], dt.bfloat16, tag='sqscr')[:Pp, :D]
            ssq = stat.tile([P, 1], dt.float32, tag='ssq')[:Pp]
            nc.scalar.activation(sq[:], src[:], AF.Square, accum_out=ssq[:])
            mm = stat.tile([P, 1], dt.float32, tag='mm')[:Pp]
            nc.vector.tensor_scalar(mm[:], s[:], 1.0 / (D * D), s[:],
                                    OP.mult, OP.mult)
            var = stat.tile([P, 1], dt.float32, tag='var')[:Pp]
            nc.vector.tensor_scalar(var[:], ssq[:], 1.0 / D, mm[:],
                                    OP.mult, OP.subtract)
            sd = stat.tile([P, 1], dt.float32, tag='sd')[:Pp]
            nc.scalar.activation(sd[:], var[:], AF.Sqrt, bias=eps_t[:Pp])
            r = stat.tile([P, 1], dt.float32, tag='r')[:Pp]
            nc.vector.reciprocal(r[:], sd[:])
            mnr = stat.tile([P, 1], dt.float32, tag='mnr')[:Pp]
            nc.vector.tensor_scalar(mnr[:], s[:], -1.0 / D, r[:],
                                    OP.mult, OP.mult)
            if gam is None:
                nc.scalar.activation(out[:], src[:], AF.Identity,
                                     bias=mnr[:], scale=r[:])
            else:
                nc.scalar.activation(scr[:], src[:], AF.Identity,
                                     bias=mnr[:], scale=r[:])
                nc.vector.tensor_mul(scr[:], scr[:], gam[:Pp, :D])
                nc.vector.tensor_add(out[:], scr[:], bet[:Pp, :D])

        def transpose6(src, dst, fp32=False):
            """src [128, 768] -> dst [128, 6, 128] bf16 via PE transposes."""
            ident = ident_f if fp32 else ident_b
            dtp = dt.float32 if fp32 else dt.bfloat16
            for ko in range(KO):
                pt = t1([P, P], dtp)
                nc.tensor.transpose(pt[:], src[:, ko * P:(ko + 1) * P], ident[:])
                nc.vector.tensor_copy(dst[:, ko, :], pt[:])

        def mm768(pt, lhsT, rhs_w, ko, start, stop):
            """accumulate lhsT.T@rhs into [128,768] psum, 512+256 bank halves."""
            nc.tensor.matmul(pt[:, 0:512], lhsT, rhs_w[:, ko, 0:512],
                             start=start, stop=stop)
            nc.tensor.matmul(pt[:, 512:768], lhsT, rhs_w[:, ko, 512:768],
                             start=start, stop=stop)

        def proj_xT(xh, w_r3, dst_kt, sl_off, bias_part):
            """dst_kt [128, KO, NSL] bf16 = (w^T @ x_std^T)[:, slice] + bias."""
            for co in range(KO):
                pt = t1([P, NSL])
                for ko in range(KO):
                    nc.tensor.matmul(
                        pt[:], w_r3[:, ko, co * P:(co + 1) * P],
                        xh[:, ko, sl_off:sl_off + NSL],
                        start=(ko == 0), stop=(ko == KO - 1))
                nc.vector.tensor_scalar_add(dst_kt[:, co, :], pt[:],
                                            W[bias_part][:, co:co + 1])

        def qT_like(w_key, b_key, rhs_pgtT, dst):
            """dst [128, KO, 128] bf16 = w^T @ pgtT (pair-stacked) + bias."""
            wt = wstream.tile([P, KO, C], dt.bfloat16, tag='ws')
            nc.sync.dma_start(wt[:], wdram[w_key][:])
            for co in range(KO):
                pt = t1([P, P])
                for ko in range(KO):
                    nc.tensor.matmul(pt[:], wt[:, ko, co * P:(co + 1) * P],
                                     rhs_pgtT[:, ko, :],
                                     start=(ko == 0), stop=(ko == KO - 1))
                nc.vector.tensor_scalar_add(dst[:, co, :], pt[:],
                                            W[b_key][:, co:co + 1])

        def mlp_pair(src, w1_key, b1_key, w2_key, bc2_key, res, out, scr, sT, hT):
            """out = res + w2^T gelu(w1^T std(src) + b1) + bc2 (pair-stacked)."""
            std_apply(src, C, scr)
            transpose6(scr, sT, fp32=True)
            for q4 in range(4):
                wt = wstream.tile([P, KO, C], dt.bfloat16, tag='ws')
                nc.sync.dma_start(wt[:], wdram[w1_key][:, :, q4 * C:(q4 + 1) * C])
                for m6 in range(KO):
                    mc = q4 * KO + m6
                    pt = t1([P, P])
                    for ko in range(KO):
                        nc.tensor.matmul(pt[:], wt[:, ko, m6 * P:(m6 + 1) * P],
                                         sT[:, ko, :],
                                         start=(ko == 0), stop=(ko == KO - 1))
                    nc.scalar.activation(hT[:, mc, :], pt[:], AF.Gelu,
                                         bias=W[b1_key][:, mc:mc + 1])
            po = t2()
            for q4 in range(4):
                wt = wstream.tile([P, KO, C], dt.bfloat16, tag='ws')
                nc.sync.dma_start(wt[:], wdram[w2_key][:, q4 * KO:(q4 + 1) * KO, :])
                for k6 in range(KO):
                    kc = q4 * KO + k6
                    mm768(po, hT[:, kc, :], wt, k6,
                          start=(kc == 0), stop=(kc == 23))
            nc.vector.tensor_add(out[:], po[:], res[:])
            nc.vector.tensor_add(out[:], out[:], W[bc2_key][:])

        # ================= main program =================
        xs_d = [dram.tile([N, C], dt.bfloat16, tag='xs%d' % b, name='xs%d' % b)
                for b in range(BL)]

        def passA(b):
            for t in range(NCH):
                xt = xin.tile([P, C], dt.float32, tag='xt')
                nc.sync.dma_start(xt[:], x_l[b, t * P:(t + 1) * P, :])
                xb = xbfp.tile([P, C], dt.bfloat16, tag='xb')
                std_apply(xt, C, xb)
                nc.sync.dma_start(xs_d[b][t * P:(t + 1) * P, :], xb[:])

        def load_xnt(b, q):
            xh = xntp.tile([P, KO, NQ], dt.bfloat16, tag='xnt')
            for ko in range(KO):
                nc.sync.dma_start_transpose(
                    xh[:, ko, :],
                    xs_d[b][q * NQ:(q + 1) * NQ, ko * P:(ko + 1) * P])
            return xh

        for p in range(2):
            bA, bB = 2 * p, 2 * p + 1
            for b in (bA, bB):
                passA(b)

            # ---- pair-early: group tokens -> pgt, qT, qblk ----
            pgt_pre = pairp.tile([P, C], dt.float32, tag='fa')
            for bb, b in enumerate((bA, bB)):
                gt_t = xin.tile([P, C], dt.float32, tag='xt')
                nc.sync.dma_start(gt_t[:], gt_l[b, :, :])
                gl = pairp.tile([P, C], dt.bfloat16, tag='gtln')
                scr = pairp.tile([P, C], dt.float32, tag='scrA')
                std_apply(gt_t, C, gl, gam=W['g_tok'], bet=W['bt_tok'], scr=scr)
                tT = pairp.tile([P, 3, C], dt.bfloat16, tag='tT')
                for mc in range(3):
                    pt = t2()
                    for half, lo, ln in ((0, 0, 512), (1, 512, 256)):
                        nc.tensor.matmul(pt[:, lo:lo + ln],
                                         W['w_i1'][:, mc * P:(mc + 1) * P],
                                         gl[:, lo:lo + ln], start=True, stop=True)
                    nc.scalar.activation(tT[:, mc, :], pt[:], AF.Gelu,
                                         bias=W['b_i1'][:, mc:mc + 1])
                ppb = t2()
                for ko in range(3):
                    for lo, ln in ((0, 512), (512, 256)):
                        nc.tensor.matmul(ppb[64 * bb:64 * bb + 64, lo:lo + ln],
                                         W['w_i2'][:, ko, :],
                                         tT[:, ko, lo:lo + ln],
                                         start=(ko == 0), stop=(ko == 2),
                                         tile_position=(0, 64 * bb))
                nc.vector.tensor_scalar_add(pgt_pre[64 * bb:64 * bb + 64, :],
                                            ppb[64 * bb:64 * bb + 64, :],
                                            W['b_i2'][64 * bb:64 * bb + 64, 0:1])
            pgt_f = pairp.tile([P, C], dt.float32, tag='fb')
            scr = pairp.tile([P, C], dt.float32, tag='scrA')
            std_apply(pgt_pre, C, pgt_f, gam=W['g_pt'], bet=W['bt_pt'], scr=scr)
            pgtT = pairp.tile([P, KO, P], dt.bfloat16, tag='pTt')
            transpose6(pgt_f, pgtT, fp32=True)
            qT = pairp.tile([P, KO, P], dt.bfloat16, tag='qTt')
            qT_like('w_q', 'b_q', pgtT, qT)
            qblk = {}
            for bb in (0, 1):
                qb = pairp.tile([P, KO, P], dt.bfloat16, tag='qblk%d' % bb)
                nc.gpsimd.memset(qb[:], 0.0)
                for j in range(KO):
                    nc.vector.tensor_copy(qb[0:64, j, 0:64],
                                          qT[0:64, j, 64 * bb:64 * bb + 64])
                    nc.vector.tensor_copy(qb[64:128, j, 64:128],
                                          qT[64:128, j, 64 * bb:64 * bb + 64])
                qblk[bb] = qb

            # ---- attention scans ----
            o_pair = pairp.tile([P, KO, P], dt.bfloat16, tag='oTt')
            for bb, b in enumerate((bA, bB)):
                ps0 = psS.tile([1, 384], dt.float32, tag='TS')
                ps1 = psS.tile([1, 384], dt.float32, tag='TS')
                o_acc = acc.tile([P, KO, HD], dt.float32, tag='o_acc')
                for sl in range(8):
                    xh = load_xnt(b, sl)
                    if True:
                        kt = ktp.tile([P, KO, NSL], dt.bfloat16, tag='kt')
                        proj_xT(xh, W['w_k'], kt, 0, 'b_k')
                        for t4 in range(4):
                            tg = sl * 4 + t4
                            lo = t4 * P
                            pv = t2()
                            for ko in range(KO):
                                mm768(pv, xh[:, ko, lo:lo + P],
                                      W['w_v'], ko, start=(ko == 0),
                                      stop=(ko == KO - 1))
                            vt = chk.tile([P, C], dt.bfloat16, tag='vt')
                            nc.vector.tensor_add(vt[:], pv[:], W['bc_v'][:])
                            at = chk.tile([P, C], dt.bfloat16, tag='at')
                            for j3 in range(2):
                                pl = t1([P, 384])
                                for jj in range(3):
                                    j = j3 * 3 + jj
                                    nc.tensor.matmul(
                                        pl[:, jj * P:(jj + 1) * P],
                                        kt[:, j, lo:lo + P], qblk[bb][:, j, :],
                                        start=True, stop=True)
                                nc.scalar.activation(
                                    at[:, j3 * 384:(j3 + 1) * 384], pl[:], AF.Exp)
                            nc.tensor.matmul(ps0[:], ones1[:], at[:, 0:384],
                                             start=(tg == 0), stop=(tg == NCH - 1))
                            nc.tensor.matmul(ps1[:], ones1[:], at[:, 384:768],
                                             start=(tg == 0), stop=(tg == NCH - 1))
                            po = t1([P, KO, HD])
                            for co in range(KO):
                                nc.tensor.matmul(
                                    po[0:64, co, :],
                                    vt[:, (2 * co) * HD:(2 * co + 1) * HD],
                                    at[:, co * P:co * P + 64],
                                    start=True, stop=True, tile_position=(0, 0))
                                nc.tensor.matmul(
                                    po[64:128, co, :],
                                    vt[:, (2 * co + 1) * HD:(2 * co + 2) * HD],
                                    at[:, co * P + 64:(co + 1) * P],
                                    start=True, stop=True, tile_position=(0, 64))
                            if tg == 0:
                                nc.vector.tensor_copy(o_acc[:], po[:])
                            else:
                                nc.vector.tensor_add(o_acc[:], o_acc[:], po[:])
                rr = row.tile([1, C], dt.float32, tag='ssb')
                nc.vector.tensor_copy(rr[:, 0:384], ps0[:])
                nc.vector.tensor_copy(rr[:, 384:768], ps1[:])
                nc.vector.reciprocal(rr[:], rr[:])
                prT = t1([P, KO, HD])
                for co in range(KO):
                    for tt in range(2):
                        hcol = (2 * co + tt) * HD
                        nc.tensor.matmul(
                            prT[64 * tt:64 * tt + 64, co, :],
                            onesf[0:1, :], rr[0:1, hcol:hcol + HD],
                            start=True, stop=True,
                            tile_position=(0, 64 * tt))
                nc.vector.tensor_mul(o_pair[:, :, 64 * bb:64 * bb + 64],
                                     o_acc[:], prT[:])

            # ---- y1 = pgt + o@pw + pb ; y-MLP ; pgt2 ; aqT ----
            py = t2()
            wp = wstream.tile([P, KO, C], dt.bfloat16, tag='ws')
            nc.sync.dma_start(wp[:], wdram['w_p'][:])
            for co in range(KO):
                mm768(py, o_pair[:, co, :], wp, co,
                      start=(co == 0), stop=(co == KO - 1))
            y1 = pairp.tile([P, C], dt.float32, tag='fa')
            nc.vector.tensor_add(y1[:], py[:], pgt_f[:])
            nc.vector.tensor_add(y1[:], y1[:], W['bc_p'][:])
            y2 = pairp.tile([P, C], dt.float32, tag='fb')
            scr = pairp.tile([P, C], dt.float32, tag='scrA')
            sT = pairp.tile([P, KO, P], dt.bfloat16, tag='sT')
            hT = pairp.tile([P, 24, P], dt.bfloat16, tag='hT')
            mlp_pair(y1, 'w_m1', 'b_m1', 'w_m2', 'bc_m2', y1, y2, scr, sT, hT)
            pgt2_f = pairp.tile([P, C], dt.float32, tag='fa')
            std_apply(y2, C, pgt2_f, gam=W['g_lnp'], bet=W['bt_lnp'], scr=scr)
            pgt2T = pairp.tile([P, KO, P], dt.bfloat16, tag='pTt')
            transpose6(pgt2_f, pgt2T, fp32=True)
            aqT = pairp.tile([P, KO, P], dt.bfloat16, tag='qTt')
            qT_like('w_asq', 'b_asq', pgt2T, aqT)

            # ---- assignment scans ----
            nxT_pair = pairp.tile([P, KO, P], dt.bfloat16, tag='oTt')
            rc_pair = acc.tile([P, 1], dt.float32, tag='rc_pair')
            for bb, b in enumerate((bA, bB)):
                nx_acc = acc.tile([P, 7, HD], dt.float32, tag='nx_acc')
                for sl in range(8):
                    xh = load_xnt(b, sl)
                    if True:
                        kt = ktp.tile([P, KO, NSL], dt.bfloat16, tag='kt')
                        proj_xT(xh, W['w_ak'], kt, 0, 'b_ak')
                        for t4 in range(4):
                            tg = sl * 4 + t4
                            lo = t4 * P
                            pv = t2()
                            for ko in range(KO):
                                mm768(pv, xh[:, ko, lo:lo + P],
                                      W['w_av'], ko, start=(ko == 0),
                                      stop=(ko == KO - 1))
                            vt = chk.tile([P, C], dt.bfloat16, tag='vt')
                            nc.vector.tensor_add(vt[:], pv[:], W['bc_av'][:])
                            pr = t1([P, HD])
                            for ko in range(KO):
                                nc.tensor.matmul(
                                    pr[:], kt[:, ko, lo:lo + P],
                                    aqT[:, ko, 64 * bb:64 * bb + 64],
                                    start=(ko == 0), stop=(ko == KO - 1))
                            rmax = stat.tile([P, 1], dt.float32, tag='rmax')
                            nc.vector.reduce_max(rmax[:], pr[:], axis=AX.X)
                            oh = chk.tile([P, HD], dt.bfloat16, tag='oh')
                            nc.vector.tensor_scalar(oh[:], pr[:], rmax[:], None,
                                                    OP.is_ge)
                            pn = t1([P, 7, HD])
                            for cc in range(KO):
                                nc.tensor.matmul(pn[:, cc, :],
                                                 vt[:, cc * P:(cc + 1) * P],
                                                 oh[:], start=True, stop=True)
                            nc.tensor.matmul(pn[:, 6, :], onesq[:], oh[:],
                                             start=True, stop=True)
                            if tg == 0:
                                nc.vector.tensor_copy(nx_acc[:], pn[:])
                            else:
                                nc.vector.tensor_add(nx_acc[:], nx_acc[:], pn[:])
                nc.vector.tensor_copy(nxT_pair[:, :, 64 * bb:64 * bb + 64],
                                      nx_acc[:, 0:6, :])
                rc1 = row.tile([1, HD], dt.float32, tag='cnt1')
                nc.vector.tensor_scalar_add(rc1[:], nx_acc[0:1, 6, :], 1.0)
                nc.vector.reciprocal(rc1[:], rc1[:])
                ptc = t1([HD, 1])
                nc.tensor.matmul(ptc[:], rc1[:], ident_f[0:1, 0:1],
                                 is_transpose=True)
                nc.vector.tensor_copy(rc_pair[64 * bb:64 * bb + 64, :], ptc[:])

            # ---- new_x proj + residual ; final channel MLP ----
            py2 = t2()
            wasp = wstream.tile([P, KO, C], dt.bfloat16, tag='ws')
            nc.sync.dma_start(wasp[:], wdram['w_asp'][:])
            for co in range(KO):
                mm768(py2, nxT_pair[:, co, :], wasp, co,
                      start=(co == 0), stop=(co == KO - 1))
            y3 = pairp.tile([P, C], dt.float32, tag='fb')
            nc.vector.tensor_scalar(y3[:], py2[:], rc_pair[:], None, OP.mult)
            nc.vector.tensor_add(y3[:], y3[:], pgt2_f[:])
            nc.vector.tensor_add(y3[:], y3[:], W['bc_asp'][:])
            outf = pairp.tile([P, C], dt.float32, tag='fc')
            mlp_pair(y3, 'w_c1', 'b_c1', 'w_c2', 'bc_c2', y3, outf, scr, sT, hT)
            nc.sync.dma_start(out_l[bA, :, :], outf[0:64, :])
            nc.sync.dma_start(out_l[bB, :, :], outf[64:128, :])

    return nc


# ----------------------------------------------------------------------
# Runner: compile once, execute via PJRT (axon) with a cached jitted call.
# ----------------------------------------------------------------------
_state = {}


def _get_runner():
    if 'run' in _state:
        return _state['run']
    import jax
    import concourse.mybir as mybir
    from concourse import bass2jax
    from jax.sharding import Mesh, PartitionSpec
    from jax.experimental.shard_map import shard_map

    nc = build_nc()
    bass2jax.install_neuronx_cc_hook()

    part_name = (nc.partition_id_tensor.name
                 if nc.partition_id_tensor is not None else None)
    in_names, out_names, out_avals, zero_outs = [], [], [], []
    for alloc in nc.m.functions[0].allocations:
        if not isinstance(alloc, mybir.MemoryLocationSet):
            continue
        name = alloc.memorylocations[0].name
        if alloc.kind == 'ExternalInput':
            if name != part_name:
                in_names.append(name)
        elif alloc.kind == 'ExternalOutput':
            out_names.append(name)
            shape = tuple(alloc.tensor_shape)
            dtype = mybir.dt.np(alloc.dtype)
            out_avals.append(jax.core.ShapedArray(shape, dtype))
            zero_outs.append(np.zeros(shape, dtype))
    n_params = len(in_names)
    all_in = in_names + out_names

    if part_name is not None:
        all_in = all_in + [part_name]

    def _body(*args):
        operands = list(args)
        if part_name is not None:
            operands.append(bass2jax.partition_id_tensor())
        outs = bass2jax._bass_exec_p.bind(
            *operands, out_avals=tuple(out_avals), in_names=tuple(all_in),
            out_names=tuple(out_names), lowering_input_output_aliases=(),
            sim_require_finite=True, sim_require_nnan=True, nc=nc)
        return tuple(outs)

    devices = jax.devices()[:NCORES]
    mesh = Mesh(np.asarray(devices), ('core',))
    n_outs = len(out_names)
    sharded = jax.jit(
        shard_map(_body, mesh=mesh,
                  in_specs=(PartitionSpec('core'),) * (n_params + n_outs),
                  out_specs=(PartitionSpec('core'),) * n_outs,
                  check_rep=False),
        donate_argnums=tuple(range(n_params, n_params + n_outs)),
        keep_unused=True)

    def run(per_core_maps):
        concat_in = [
            np.concatenate([np.asarray(per_core_maps[c][nm])
                            for c in range(NCORES)], axis=0)
            for nm in in_names]
        concat_zeros = [np.zeros((NCORES * z.shape[0], *z.shape[1:]), z.dtype)
                        for z in zero_outs]
        outs = sharded(*concat_in, *concat_zeros)
        res = np.asarray(outs[0]).reshape(NCORES, *out_avals[0].shape)
        return res

    _state['run'] = run
    _state['nc'] = nc
    _state['in_names'] = in_names
    return run


def make_in_maps(inputs):
    w = prep_weights(inputs)
    x = np.ascontiguousarray(np.asarray(inputs['x'], F32)
                             .reshape(NCORES, BL, N, C))
    gt = np.ascontiguousarray(np.asarray(inputs['group_tokens'], F32)
                              .reshape(NCORES, BL, G_IN, C))
    maps = []
    for c in range(NCORES):
        m = dict(w)
        m['x_l'] = x[c]
        m['gt_l'] = gt[c]
        maps.append(m)
    return maps


def kernel(**inputs):
    run = _get_runner()
    res = run(make_in_maps(inputs))          # [8, BL, 64, 768]
    return np.ascontiguousarray(res.reshape(B, G_OUT, C).astype(np.float32))


# revision 18
# speedup vs baseline: 134.8011x; 134.8011x over previous
"""GroupingBlock Bass/Tile kernel for 8 Trainium2 NeuronCores.

Data-parallel over batch B=32 -> 4 batches/core, weights replicated.
Host preprocessing folds LN gammas/betas into adjacent matmul weights where
algebraically exact, pre-broadcasts free-dim biases, casts matmul operands
to bf16, and lays weights out as [128, K//128, N] kxm tiles.

On-chip per batch: standardize x (per-token mean/rstd) -> bf16 -> DRAM
roundtrip with an XBAR DMA-transpose to get x_std^T [768, 4096]; all
projections contract over channels with x_std^T as an operand. Attention
runs in transposed layout (logits^T [n, heads*g]): softmax normalizers come
from ones-matmuls (no max subtraction needed; logits are O(1)), the
unnormalized exp weights feed attn@v directly, and 1/sum is applied to the
accumulated output. The hard assignment is a free-dim reduce_max + is_ge
mask (exact one-hot up to measure-zero ties).
"""

import os
import sys
import numpy as np
import ml_dtypes

if '/opt/trn_rl_repo' not in sys.path:
    sys.path.insert(0, '/opt/trn_rl_repo')

B, N, G_IN, G_OUT, C, H = 32, 4096, 128, 64, 768, 12
TOK_HID, MLP_HID, CH_HID = 384, 3072, 3072
NCORES = 8
BL = B // NCORES          # 4 batches per core
P = 128
KO = C // P               # 6 contraction chunks over channels
HD = C // H               # 64 head dim
NCH = N // P              # 32 n-chunks per batch
NSL = 512                 # n-slice for kT production
NQ = 512                  # xnT slice size
F32 = np.float32
BF16 = ml_dtypes.bfloat16


def _r3(w):
    """[C_in, M] -> [128, C_in//128, M] kxm layout, bf16."""
    return np.ascontiguousarray(
        w.reshape(-1, P, w.shape[-1]).transpose(1, 0, 2).astype(BF16))


def _bias_part(b):
    """[D] -> [128, D//128] per-partition bias layout (f32)."""
    return np.ascontiguousarray(b.reshape(-1, P).T.astype(F32))


def _bc(b):
    """[D] -> [128, D] broadcast (bf16)."""
    return np.ascontiguousarray(
        np.broadcast_to(b.astype(F32), (P, len(b))).astype(BF16))


def prep_weights(inp):
    """Host-side preprocessing -> dict of staged arrays (shared by cores)."""
    g = lambda k: np.asarray(inp[k], F32)
    w = {}
    lxg, lxb = g('ln_x_g'), g('ln_x_b')
    for nm, wk, bk in (('k', 'ca_kw', 'ca_kb'), ('v', 'ca_vw', 'ca_vb'),
                       ('ak', 'as_kw', 'as_kb'), ('av', 'as_vw', 'as_vb')):
        W = g(wk) * lxg[:, None]
        be = g(bk) + lxb @ g(wk)
        w['w_' + nm] = _r3(W)
        if nm in ('k', 'ak'):
            w['b_' + nm] = _bias_part(be)
        else:
            w['bc_' + nm] = _bc(be)
    s1 = float(HD) ** -0.5
    w['w_q'] = _r3(g('ca_qw') * s1)
    w['b_q'] = _bias_part(g('ca_qb') * s1)
    s2 = float(C) ** -0.5
    w['w_asq'] = _r3(g('as_qw') * s2)
    w['b_asq'] = _bias_part(g('as_qb') * s2)
    w['w_p'] = _r3(g('ca_pw'))
    w['bc_p'] = _bc(g('ca_pb'))
    w['w_asp'] = _r3(g('as_pw'))
    w['bc_asp'] = _bc(g('as_pb'))
    w['w_i1'] = np.ascontiguousarray(g('inter_w1').astype(BF16))
    w['b_i1'] = _bias_part(g('inter_b1'))
    w['w_i2'] = _r3(g('inter_w2'))
    w['b_i2'] = np.ascontiguousarray(
        np.tile(g('inter_b2'), 2)[:, None].astype(F32))
    for nm, lng, lnb, w1k, b1k, w2k, b2k in (
            ('m', 'ca_ln2_g', 'ca_ln2_b', 'ca_m1w', 'ca_m1b', 'ca_m2w', 'ca_m2b'),
            ('c', 'ln_nx_g', 'ln_nx_b', 'mc_w1', 'mc_b1', 'mc_w2', 'mc_b2')):
        W1 = g(w1k) * g(lng)[:, None]
        b1 = g(b1k) + g(lnb) @ g(w1k)
        w['w_%s1' % nm] = _r3(W1)
        w['b_%s1' % nm] = _bias_part(b1)
        w['w_%s2' % nm] = _r3(g(w2k))
        w['bc_%s2' % nm] = _bc(g(b2k))
    for nm, gk, bk in (('tok', 'ln_tokens_g', 'ln_tokens_b'),
                       ('pt', 'ln_pt_g', 'ln_pt_b'),
                       ('lnp', 'ca_lnp_g', 'ca_lnp_b')):
        w['g_' + nm] = _bc(g(gk))
        w['bt_' + nm] = _bc(g(bk))
    return w


def build_nc():
    """Build the Bass module for one core."""
    import concourse.bass as bass
    import concourse.tile as tile
    import concourse.mybir as mybir
    from concourse.masks import make_identity
    from contextlib import ExitStack

    dt = mybir.dt
    AF = mybir.ActivationFunctionType
    OP = mybir.AluOpType
    AX = mybir.AxisListType

    nc = bass.Bass('TRN2', target_bir_lowering=False, debug=False,
                   num_devices=NCORES)

    x_l = nc.dram_tensor('x_l', (BL, N, C), dt.float32, kind='ExternalInput').ap()
    gt_l = nc.dram_tensor('gt_l', (BL, G_IN, C), dt.float32,
                          kind='ExternalInput').ap()
    wspec = [
        ('w_k', (P, KO, C), dt.bfloat16), ('w_v', (P, KO, C), dt.bfloat16),
        ('w_ak', (P, KO, C), dt.bfloat16), ('w_av', (P, KO, C), dt.bfloat16),
        ('b_k', (P, KO), dt.float32), ('b_ak', (P, KO), dt.float32),
        ('bc_v', (P, C), dt.bfloat16), ('bc_av', (P, C), dt.bfloat16),
        ('w_q', (P, KO, C), dt.bfloat16), ('b_q', (P, KO), dt.float32),
        ('w_asq', (P, KO, C), dt.bfloat16), ('b_asq', (P, KO), dt.float32),
        ('w_p', (P, KO, C), dt.bfloat16), ('bc_p', (P, C), dt.bfloat16),
        ('w_asp', (P, KO, C), dt.bfloat16), ('bc_asp', (P, C), dt.bfloat16),
        ('w_i1', (P, TOK_HID), dt.bfloat16), ('b_i1', (P, 3), dt.float32),
        ('w_i2', (P, 3, G_OUT), dt.bfloat16), ('b_i2', (P, 1), dt.float32),
        ('w_m1', (P, KO, MLP_HID), dt.bfloat16), ('b_m1', (P, 24), dt.float32),
        ('w_m2', (P, 24, C), dt.bfloat16), ('bc_m2', (P, C), dt.bfloat16),
        ('w_c1', (P, KO, CH_HID), dt.bfloat16), ('b_c1', (P, 24), dt.float32),
        ('w_c2', (P, 24, C), dt.bfloat16), ('bc_c2', (P, C), dt.bfloat16),
        ('g_tok', (P, C), dt.bfloat16), ('bt_tok', (P, C), dt.bfloat16),
        ('g_pt', (P, C), dt.bfloat16), ('bt_pt', (P, C), dt.bfloat16),
        ('g_lnp', (P, C), dt.bfloat16), ('bt_lnp', (P, C), dt.bfloat16),
    ]
    wdram = {nm: nc.dram_tensor(nm, shp, d, kind='ExternalInput').ap()
             for nm, shp, d in wspec}
    out_l = nc.dram_tensor('out_l', (BL, G_OUT, C), dt.float32,
                           kind='ExternalOutput').ap()

    RESIDENT = ['w_k', 'w_v', 'w_ak', 'w_av', 'b_k', 'b_ak', 'bc_v', 'bc_av',
                'b_q', 'b_asq', 'bc_p', 'bc_asp', 'w_i1', 'b_i1', 'w_i2',
                'b_i2', 'b_m1', 'bc_m2', 'b_c1', 'bc_c2',
                'g_tok', 'bt_tok', 'g_pt', 'bt_pt', 'g_lnp', 'bt_lnp']

    class _NP:
        """tile_pool wrapper that auto-names tiles (name inference fails
        when the tile is sliced on the allocation line)."""
        _i = [0]

        def __init__(self, pool):
            self._pool = pool

        def tile(self, shape, dtype, tag, name=None):
            self._i[0] += 1
            return self._pool.tile(shape, dtype, tag=tag,
                                   name=name or '%s_%d' % (tag, self._i[0]))

    def _install_wait_splitter(tc):
        # The neuronxcc walrus in this container accepts at most ONE sync
        # wait per instruction; Tile emits several. Split extras into
        # standalone EventSemaphore ops (same engine, immediately before),
        # and split the tail drain's waits across multiple drains.
        orig_add = tc._add_instruction
        seq = [0]

        def patched_add(inst):
            si = inst.sync_info
            if si is not None and len(si.on_wait) > 1:
                waits = list(si.on_wait)
                for w in waits[:-1]:
                    seq[0] += 1
                    es = mybir.InstEventSemaphore(
                        name='I-ws%d' % seq[0], ins=[], outs=[],
                        engine=inst.engine)
                    es.sync_info = mybir.SyncInfo(on_wait=[w], on_update=[])
                    orig_add(es)
                inst.sync_info = mybir.SyncInfo(on_wait=waits[-1:],
                                                on_update=list(si.on_update))
            orig_add(inst)

        tc._add_instruction = patched_add

        from concourse.vector_clock import ScopedClock

        def patched_dab(tick_clock, wait_clock):
            drain_bi = nc.sync.drain()
            wait_clock.add_sem_waits(
                drain_bi.ins, ScopedClock({None: tick_clock.global_clock}))
            mi = drain_bi.ins
            si = mi.sync_info
            if si is not None and len(si.on_wait) > 1:
                waits = list(si.on_wait)
                mi.sync_info = mybir.SyncInfo(on_wait=waits[:1],
                                              on_update=list(si.on_update))
                for w in waits[1:]:
                    d2 = nc.sync.drain()
                    d2.ins.sync_info = mybir.SyncInfo(on_wait=[w], on_update=[])
            nc.all_engine_barrier()
            assert tc.sems is not None
            popped = nc._tile_sem_poison_stack.pop()
            assert popped is tc._sem_poison
            nc.clear_and_free_semaphores(list(tc.sems.allocated().values()))
            nc.all_engine_barrier()

        tc._drain_and_barrier = patched_dab

    with tile.TileContext(nc) as tc, ExitStack() as ctx:
        _install_wait_splitter(tc)
        konst = ctx.enter_context(tc.tile_pool(name='konst', bufs=1))
        wstream = ctx.enter_context(tc.tile_pool(name='wstream', bufs=2))
        dram = ctx.enter_context(tc.tile_pool(name='dram', bufs=1, space='DRAM'))
        xin = ctx.enter_context(tc.tile_pool(name='xin', bufs=2))
        xbfp = ctx.enter_context(tc.tile_pool(name='xbfp', bufs=2))
        stat = ctx.enter_context(tc.tile_pool(name='stat', bufs=4))
        row = ctx.enter_context(tc.tile_pool(name='row', bufs=2))
        xntp = ctx.enter_context(tc.tile_pool(name='xntp', bufs=2))
        ktp = ctx.enter_context(tc.tile_pool(name='ktp', bufs=2))
        chk = ctx.enter_context(tc.tile_pool(name='chk', bufs=2))
        acc = ctx.enter_context(tc.tile_pool(name='acc', bufs=1))
        pairp = ctx.enter_context(tc.tile_pool(name='pairp', bufs=1))
        psA = ctx.enter_context(tc.tile_pool(name='psA', bufs=3, space='PSUM'))
        psB = ctx.enter_context(tc.tile_pool(name='psB', bufs=1, space='PSUM'))
        psS = ctx.enter_context(tc.tile_pool(name='psS', bufs=2, space='PSUM'))
        konst, wstream, dram, row = _NP(konst), _NP(wstream), _NP(dram), _NP(row)
        xin, xbfp, stat, xntp = _NP(xin), _NP(xbfp), _NP(stat), _NP(xntp)
        ktp, chk, acc, pairp = _NP(ktp), _NP(chk), _NP(acc), _NP(pairp)
        psA, psB, psS = _NP(psA), _NP(psB), _NP(psS)

        W = {}
        for nm in RESIDENT:
            t = konst.tile(list(wdram[nm].shape), wdram[nm].dtype, tag='k_' + nm)
            nc.sync.dma_start(t[:], wdram[nm][:])
            W[nm] = t
        ident_b = konst.tile([P, P], dt.bfloat16, tag='ident_b')
        make_identity(nc, ident_b[:])
        ident_f = konst.tile([P, P], dt.float32, tag='ident_f')
        make_identity(nc, ident_f[:])
        ones1 = konst.tile([P, 1], dt.bfloat16, tag='ones1')
        nc.vector.memset(ones1[:], 1.0)
        onesq = konst.tile([P, P], dt.bfloat16, tag='onesq')
        nc.vector.memset(onesq[:], 1.0)
        eps_t = konst.tile([P, 1], dt.float32, tag='eps_t')
        nc.vector.memset(eps_t[:], 1e-5)
        onesf = konst.tile([P, HD], dt.float32, tag='onesf')
        nc.vector.memset(onesf[:], 1.0)

        def t1(shape, dtype=dt.float32):
            return psA.tile(shape, dtype, tag='T1')

        def t2():
            return psB.tile([P, C], dt.float32, tag='T2')

        def std_apply(src, D, out, gam=None, bet=None, scr=None):
            """out = standardize(src) [* gam + bet]; src [Pp, D] f32."""
            Pp = src.shape[0]
            s = stat.tile([P, 1], dt.float32, tag='s')[:Pp]
            nc.vector.reduce_sum(s[:], src[:], axis=AX.X)
            sq = xbfp.tile([P, C], dt.bfloat16, tag='sqscr')[:Pp, :D]
            ssq = stat.tile([P, 1], dt.float32, tag='ssq')[:Pp]
            nc.scalar.activation(sq[:], src[:], AF.Square, accum_out=ssq[:])
            mm = stat.tile([P, 1], dt.float32, tag='mm')[:Pp]
            nc.vector.tensor_scalar(mm[:], s[:], 1.0 / (D * D), s[:],
                                    OP.mult, OP.mult)
            var = stat.tile([P, 1], dt.float32, tag='var')[:Pp]
            nc.vector.tensor_scalar(var[:], ssq[:], 1.0 / D, mm[:],
                                    OP.mult, OP.subtract)
            sd = stat.tile([P, 1], dt.float32, tag='sd')[:Pp]
            nc.scalar.activation(sd[:], var[:], AF.Sqrt, bias=eps_t[:Pp])
            r = stat.tile([P, 1], dt.float32, tag='r')[:Pp]
            nc.vector.reciprocal(r[:], sd[:])
            mnr = stat.tile([P, 1], dt.float32, tag='mnr')[:Pp]
            nc.vector.tensor_scalar(mnr[:], s[:], -1.0 / D, r[:],
                                    OP.mult, OP.mult)
            if gam is None:
                nc.scalar.activation(out[:], src[:], AF.Identity,
                                     bias=mnr[:], scale=r[:])
            else:
                nc.scalar.activation(scr[:], src[:], AF.Identity,
                                     bias=mnr[:], scale=r[:])
                nc.vector.tensor_mul(scr[:], scr[:], gam[:Pp, :D])
                nc.vector.tensor_add(out[:], scr[:], bet[:Pp, :D])

        def transpose6(src, dst, fp32=False):
            """src [128, 768] -> dst [128, 6, 128] bf16 via PE transposes."""
            ident = ident_f if fp32 else ident_b
            dtp = dt.float32 if fp32 else dt.bfloat16
            for ko in range(KO):
                pt = t1([P, P], dtp)
                nc.tensor.transpose(pt[:], src[:, ko * P:(ko + 1) * P], ident[:])
                nc.vector.tensor_copy(dst[:, ko, :], pt[:])

        def mm768(pt, lhsT, rhs_w, ko, start, stop):
            """accumulate lhsT.T@rhs into [128,768] psum, 512+256 bank halves."""
            nc.tensor.matmul(pt[:, 0:512], lhsT, rhs_w[:, ko, 0:512],
                             start=start, stop=stop)
            nc.tensor.matmul(pt[:, 512:768], lhsT, rhs_w[:, ko, 512:768],
                             start=start, stop=stop)

        def proj_xT(xh, w_r3, dst_kt, sl_off, bias_part):
            """dst_kt [128, KO, NSL] bf16 = (w^T @ x_std^T)[:, slice] + bias."""
            for co in range(KO):
                pt = t1([P, NSL])
                for ko in range(KO):
                    nc.tensor.matmul(
                        pt[:], w_r3[:, ko, co * P:(co + 1) * P],
                        xh[:, ko, sl_off:sl_off + NSL],
                        start=(ko == 0), stop=(ko == KO - 1))
                nc.vector.tensor_scalar_add(dst_kt[:, co, :], pt[:],
                                            W[bias_part][:, co:co + 1])

        def qT_like(w_key, b_key, rhs_pgtT, dst):
            """dst [128, KO, 128] bf16 = w^T @ pgtT (pair-stacked) + bias."""
            wt = wstream.tile([P, KO, C], dt.bfloat16, tag='ws')
            nc.sync.dma_start(wt[:], wdram[w_key][:])
            for co in range(KO):
                pt = t1([P, P])
                for ko in range(KO):
                    nc.tensor.matmul(pt[:], wt[:, ko, co * P:(co + 1) * P],
                                     rhs_pgtT[:, ko, :],
                                     start=(ko == 0), stop=(ko == KO - 1))
                nc.vector.tensor_scalar_add(dst[:, co, :], pt[:],
                                            W[b_key][:, co:co + 1])

        def mlp_pair(src, w1_key, b1_key, w2_key, bc2_key, res, out, scr, sT, hT):
            """out = res + w2^T gelu(w1^T std(src) + b1) + bc2 (pair-stacked)."""
            std_apply(src, C, scr)
            transpose6(scr, sT, fp32=True)
            for q4 in range(4):
                wt = wstream.tile([P, KO, C], dt.bfloat16, tag='ws')
                nc.sync.dma_start(wt[:], wdram[w1_key][:, :, q4 * C:(q4 + 1) * C])
                for m6 in range(KO):
                    mc = q4 * KO + m6
                    pt = t1([P, P])
                    for ko in range(KO):
                        nc.tensor.matmul(pt[:], wt[:, ko, m6 * P:(m6 + 1) * P],
                                         sT[:, ko, :],
                                         start=(ko == 0), stop=(ko == KO - 1))
                    nc.scalar.activation(hT[:, mc, :], pt[:], AF.Gelu,
                                         bias=W[b1_key][:, mc:mc + 1])
            po = t2()
            for q4 in range(4):
                wt = wstream.tile([P, KO, C], dt.bfloat16, tag='ws')
                nc.sync.dma_start(wt[:], wdram[w2_key][:, q4 * KO:(q4 + 1) * KO, :])
                for k6 in range(KO):
                    kc = q4 * KO + k6
                    mm768(po, hT[:, kc, :], wt, k6,
                          start=(kc == 0), stop=(kc == 23))
            nc.vector.tensor_add(out[:], po[:], res[:])
            nc.vector.tensor_add(out[:], out[:], W[bc2_key][:])

        # ================= main program =================
        xs_d = [dram.tile([N, C], dt.bfloat16, tag='xs%d' % b, name='xs%d' % b)
                for b in range(BL)]

        def passA(b):
            for t in range(NCH):
                xt = xin.tile([P, C], dt.float32, tag='xt')
                nc.sync.dma_start(xt[:], x_l[b, t * P:(t + 1) * P, :])
                xb = xbfp.tile([P, C], dt.bfloat16, tag='xb')
                std_apply(xt, C, xb)
                nc.sync.dma_start(xs_d[b][t * P:(t + 1) * P, :], xb[:])

        def load_xnt(b, q):
            xh = xntp.tile([P, KO, NQ], dt.bfloat16, tag='xnt')
            for ko in range(KO):
                nc.sync.dma_start_transpose(
                    xh[:, ko, :],
                    xs_d[b][q * NQ:(q + 1) * NQ, ko * P:(ko + 1) * P])
            return xh

        for p in range(2):
            bA, bB = 2 * p, 2 * p + 1
            for b in (bA, bB):
                passA(b)

            # ---- pair-early: group tokens -> pgt, qT, qblk ----
            pgt_pre = pairp.tile([P, C], dt.float32, tag='fa')
            for bb, b in enumerate((bA, bB)):
                gt_t = xin.tile([P, C], dt.float32, tag='xt')
                nc.sync.dma_start(gt_t[:], gt_l[b, :, :])
                gl = pairp.tile([P, C], dt.bfloat16, tag='gtln')
                scr = pairp.tile([P, C], dt.float32, tag='scrA')
                std_apply(gt_t, C, gl, gam=W['g_tok'], bet=W['bt_tok'], scr=scr)
                tT = pairp.tile([P, 3, C], dt.bfloat16, tag='tT')
                for mc in range(3):
                    pt = t2()
                    for half, lo, ln in ((0, 0, 512), (1, 512, 256)):
                        nc.tensor.matmul(pt[:, lo:lo + ln],
                                         W['w_i1'][:, mc * P:(mc + 1) * P],
                                         gl[:, lo:lo + ln], start=True, stop=True)
                    nc.scalar.activation(tT[:, mc, :], pt[:], AF.Gelu,
                                         bias=W['b_i1'][:, mc:mc + 1])
                ppb = t2()
                for ko in range(3):
                    for lo, ln in ((0, 512), (512, 256)):
                        nc.tensor.matmul(ppb[64 * bb:64 * bb + 64, lo:lo + ln],
                                         W['w_i2'][:, ko, :],
                                         tT[:, ko, lo:lo + ln],
                                         start=(ko == 0), stop=(ko == 2),
                                         tile_position=(0, 64 * bb))
                nc.vector.tensor_scalar_add(pgt_pre[64 * bb:64 * bb + 64, :],
                                            ppb[64 * bb:64 * bb + 64, :],
                                            W['b_i2'][64 * bb:64 * bb + 64, 0:1])
            pgt_f = pairp.tile([P, C], dt.float32, tag='fb')
            scr = pairp.tile([P, C], dt.float32, tag='scrA')
            std_apply(pgt_pre, C, pgt_f, gam=W['g_pt'], bet=W['bt_pt'], scr=scr)
            pgtT = pairp.tile([P, KO, P], dt.bfloat16, tag='pTt')
            transpose6(pgt_f, pgtT, fp32=True)
            qT = pairp.tile([P, KO, P], dt.bfloat16, tag='qTt')
            qT_like('w_q', 'b_q', pgtT, qT)
            qblk = {}
            for bb in (0, 1):
                qb = pairp.tile([P, KO, P], dt.bfloat16, tag='qblk%d' % bb)
                nc.gpsimd.memset(qb[:], 0.0)
                for j in range(KO):
                    nc.vector.tensor_copy(qb[0:64, j, 0:64],
                                          qT[0:64, j, 64 * bb:64 * bb + 64])
                    nc.vector.tensor_copy(qb[64:128, j, 64:128],
                                          qT[64:128, j, 64 * bb:64 * bb + 64])
                qblk[bb] = qb

            # ---- attention scans ----
            o_pair = pairp.tile([P, KO, P], dt.bfloat16, tag='oTt')
            for bb, b in enumerate((bA, bB)):
                ps0 = psS.tile([1, 384], dt.float32, tag='TS')
                ps1 = psS.tile([1, 384], dt.float32, tag='TS')
                o_acc = acc.tile([P, KO, HD], dt.float32, tag='o_acc')
                for sl in range(8):
                    xh = load_xnt(b, sl)
                    if True:
                        kt = ktp.tile([P, KO, NSL], dt.bfloat16, tag='kt')
                        proj_xT(xh, W['w_k'], kt, 0, 'b_k')
                        for t4 in range(4):
                            tg = sl * 4 + t4
                            lo = t4 * P
                            pv = t2()
                            for ko in range(KO):
                                mm768(pv, xh[:, ko, lo:lo + P],
                                      W['w_v'], ko, start=(ko == 0),
                                      stop=(ko == KO - 1))
                            vt = chk.tile([P, C], dt.bfloat16, tag='vt')
                            nc.vector.tensor_add(vt[:], pv[:], W['bc_v'][:])
                            at = chk.tile([P, C], dt.bfloat16, tag='at')
                            for j3 in range(2):
                                pl = t1([P, 384])
                                for jj in range(3):
                                    j = j3 * 3 + jj
                                    nc.tensor.matmul(
                                        pl[:, jj * P:(jj + 1) * P],
                                        kt[:, j, lo:lo + P], qblk[bb][:, j, :],
                                        start=True, stop=True)
                                nc.scalar.activation(
                                    at[:, j3 * 384:(j3 + 1) * 384], pl[:], AF.Exp)
                            nc.tensor.matmul(ps0[:], ones1[:], at[:, 0:384],
                                             start=(tg == 0), stop=(tg == NCH - 1))
                            nc.tensor.matmul(ps1[:], ones1[:], at[:, 384:768],
                                             start=(tg == 0), stop=(tg == NCH - 1))
                            po = t1([P, KO, HD])
                            for co in range(KO):
                                nc.tensor.matmul(
                                    po[0:64, co, :],
                                    vt[:, (2 * co) * HD:(2 * co + 1) * HD],
                                    at[:, co * P:co * P + 64],
                                    start=True, stop=True, tile_position=(0, 0))
                                nc.tensor.matmul(
                                    po[64:128, co, :],
                                    vt[:, (2 * co + 1) * HD:(2 * co + 2) * HD],
                                    at[:, co * P + 64:(co + 1) * P],
                                    start=True, stop=True, tile_position=(0, 64))
                            if tg == 0:
                                nc.vector.tensor_copy(o_acc[:], po[:])
                            else:
                                nc.vector.tensor_add(o_acc[:], o_acc[:], po[:])
                rr = row.tile([1, C], dt.float32, tag='ssb')
                nc.vector.tensor_copy(rr[:, 0:384], ps0[:])
                nc.vector.tensor_copy(rr[:, 384:768], ps1[:])
                nc.vector.reciprocal(rr[:], rr[:])
                prT = t1([P, KO, HD])
                for co in range(KO):
                    for tt in range(2):
                        hcol = (2 * co + tt) * HD
                        nc.tensor.matmul(
                            prT[64 * tt:64 * tt + 64, co, :],
                            onesf[0:1, :], rr[0:1, hcol:hcol + HD],
                            start=True, stop=True,
                            tile_position=(0, 64 * tt))
                nc.vector.tensor_mul(o_pair[:, :, 64 * bb:64 * bb + 64],
                                     o_acc[:], prT[:])

            # ---- y1 = pgt + o@pw + pb ; y-MLP ; pgt2 ; aqT ----
            py = t2()
            wp = wstream.tile([P, KO, C], dt.bfloat16, tag='ws')
            nc.sync.dma_start(wp[:], wdram['w_p'][:])
            for co in range(KO):
                mm768(py, o_pair[:, co, :], wp, co,
                      start=(co == 0), stop=(co == KO - 1))
            y1 = pairp.tile([P, C], dt.float32, tag='fa')
            nc.vector.tensor_add(y1[:], py[:], pgt_f[:])
            nc.vector.tensor_add(y1[:], y1[:], W['bc_p'][:])
            y2 = pairp.tile([P, C], dt.float32, tag='fb')
            scr = pairp.tile([P, C], dt.float32, tag='scrA')
            sT = pairp.tile([P, KO, P], dt.bfloat16, tag='sT')
            hT = pairp.tile([P, 24, P], dt.bfloat16, tag='hT')
            mlp_pair(y1, 'w_m1', 'b_m1', 'w_m2', 'bc_m2', y1, y2, scr, sT, hT)
            pgt2_f = pairp.tile([P, C], dt.float32, tag='fa')
            std_apply(y2, C, pgt2_f, gam=W['g_lnp'], bet=W['bt_lnp'], scr=scr)
            pgt2T = pairp.tile([P, KO, P], dt.bfloat16, tag='pTt')
            transpose6(pgt2_f, pgt2T, fp32=True)
            aqT = pairp.tile([P, KO, P], dt.bfloat16, tag='qTt')
            qT_like('w_asq', 'b_asq', pgt2T, aqT)

            # ---- assignment scans ----
            nxT_pair = pairp.tile([P, KO, P], dt.bfloat16, tag='oTt')
            rc_pair = acc.tile([P, 1], dt.float32, tag='rc_pair')
            for bb, b in enumerate((bA, bB)):
                nx_acc = acc.tile([P, 7, HD], dt.float32, tag='nx_acc')
                for sl in range(8):
                    xh = load_xnt(b, sl)
                    if True:
                        kt = ktp.tile([P, KO, NSL], dt.bfloat16, tag='kt')
                        proj_xT(xh, W['w_ak'], kt, 0, 'b_ak')
                        for t4 in range(4):
                            tg = sl * 4 + t4
                            lo = t4 * P
                            pv = t2()
                            for ko in range(KO):
                                mm768(pv, xh[:, ko, lo:lo + P],
                                      W['w_av'], ko, start=(ko == 0),
                                      stop=(ko == KO - 1))
                            vt = chk.tile([P, C], dt.bfloat16, tag='vt')
                            nc.vector.tensor_add(vt[:], pv[:], W['bc_av'][:])
                            pr = t1([P, HD])
                            for ko in range(KO):
                                nc.tensor.matmul(
                                    pr[:], kt[:, ko, lo:lo + P],
                                    aqT[:, ko, 64 * bb:64 * bb + 64],
                                    start=(ko == 0), stop=(ko == KO - 1))
                            rmax = stat.tile([P, 1], dt.float32, tag='rmax')
                            nc.vector.reduce_max(rmax[:], pr[:], axis=AX.X)
                            oh = chk.tile([P, HD], dt.bfloat16, tag='oh')
                            nc.vector.tensor_scalar(oh[:], pr[:], rmax[:], None,
                                                    OP.is_ge)
                            pn = t1([P, 7, HD])
                            for cc in range(KO):
                                nc.tensor.matmul(pn[:, cc, :],
                                                 vt[:, cc * P:(cc + 1) * P],
                                                 oh[:], start=True, stop=True)
                            nc.tensor.matmul(pn[:, 6, :], onesq[:], oh[:],
                                             start=True, stop=True)
                            if tg == 0:
                                nc.vector.tensor_copy(nx_acc[:], pn[:])
                            else:
                                nc.vector.tensor_add(nx_acc[:], nx_acc[:], pn[:])
                nc.vector.tensor_copy(nxT_pair[:, :, 64 * bb:64 * bb + 64],
                                      nx_acc[:, 0:6, :])
                rc1 = row.tile([1, HD], dt.float32, tag='cnt1')
                nc.vector.tensor_scalar_add(rc1[:], nx_acc[0:1, 6, :], 1.0)
                nc.vector.reciprocal(rc1[:], rc1[:])
                ptc = t1([HD, 1])
                nc.tensor.matmul(ptc[:], rc1[:], ident_f[0:1, 0:1],
                                 is_transpose=True)
                nc.vector.tensor_copy(rc_pair[64 * bb:64 * bb + 64, :], ptc[:])

            # ---- new_x proj + residual ; final channel MLP ----
            py2 = t2()
            wasp = wstream.tile([P, KO, C], dt.bfloat16, tag='ws')
            nc.sync.dma_start(wasp[:], wdram['w_asp'][:])
            for co in range(KO):
                mm768(py2, nxT_pair[:, co, :], wasp, co,
                      start=(co == 0), stop=(co == KO - 1))
            y3 = pairp.tile([P, C], dt.float32, tag='fb')
            nc.vector.tensor_scalar(y3[:], py2[:], rc_pair[:], None, OP.mult)
            nc.vector.tensor_add(y3[:], y3[:], pgt2_f[:])
            nc.vector.tensor_add(y3[:], y3[:], W['bc_asp'][:])
            outf = pairp.tile([P, C], dt.float32, tag='fc')
            mlp_pair(y3, 'w_c1', 'b_c1', 'w_c2', 'bc_c2', y3, outf, scr, sT, hT)
            nc.sync.dma_start(out_l[bA, :, :], outf[0:64, :])
            nc.sync.dma_start(out_l[bB, :, :], outf[64:128, :])

    return nc


# ----------------------------------------------------------------------
# Runner: compile once, execute via PJRT (axon) with a cached jitted call.
# ----------------------------------------------------------------------
_state = {}


def _get_runner():
    if 'run' in _state:
        return _state['run']
    import jax
    import concourse.mybir as mybir
    from concourse import bass2jax
    from jax.sharding import Mesh, PartitionSpec
    from jax.experimental.shard_map import shard_map

    nc = build_nc()
    bass2jax.install_neuronx_cc_hook()

    part_name = (nc.partition_id_tensor.name
                 if nc.partition_id_tensor is not None else None)
    in_names, out_names, out_avals, zero_outs = [], [], [], []
    for alloc in nc.m.functions[0].allocations:
        if not isinstance(alloc, mybir.MemoryLocationSet):
            continue
        name = alloc.memorylocations[0].name
        if alloc.kind == 'ExternalInput':
            if name != part_name:
                in_names.append(name)
        elif alloc.kind == 'ExternalOutput':
            out_names.append(name)
            shape = tuple(alloc.tensor_shape)
            dtype = mybir.dt.np(alloc.dtype)
            out_avals.append(jax.core.ShapedArray(shape, dtype))
            zero_outs.append(np.zeros(shape, dtype))
    n_params = len(in_names)
    all_in = in_names + out_names

    if part_name is not None:
        all_in = all_in + [part_name]

    def _body(*args):
        operands = list(args)
        if part_name is not None:
            operands.append(bass2jax.partition_id_tensor())
        outs = bass2jax._bass_exec_p.bind(
            *operands, out_avals=tuple(out_avals), in_names=tuple(all_in),
            out_names=tuple(out_names), lowering_input_output_aliases=(),
            sim_require_finite=True, sim_require_nnan=True, nc=nc)
        return tuple(outs)

    devices = jax.devices()[:NCORES]
    mesh = Mesh(np.asarray(devices), ('core',))
    n_outs = len(out_names)
    sharded = jax.jit(
        shard_map(_body, mesh=mesh,
                  in_specs=(PartitionSpec('core'),) * (n_params + n_outs),
                  out_specs=(PartitionSpec('core'),) * n_outs,
                  check_rep=False),
        donate_argnums=tuple(range(n_params, n_params + n_outs)),
        keep_unused=True)

    from jax.sharding import NamedSharding

    shard = NamedSharding(mesh, PartitionSpec('core'))
    zshapes = [(NCORES * z.shape[0], *z.shape[1:]) for z in zero_outs]
    zdtypes = [z.dtype for z in zero_outs]

    def _make_zeros():
        import jax.numpy as jnp
        return [
            jax.device_put(np.zeros(s, d), shard)
            for s, d in zip(zshapes, zdtypes)]

    def stage(per_core_maps):
        concat_in = [
            np.concatenate([np.asarray(per_core_maps[c][nm])
                            for c in range(NCORES)], axis=0)
            for nm in in_names]
        return [jax.device_put(a, shard) for a in concat_in]

    def run_staged(staged):
        outs = sharded(*staged, *_make_zeros())
        jax.block_until_ready(outs)
        return outs

    def run_staged_timed(staged):
        import time as _time
        zeros = _make_zeros()
        for z in zeros:
            jax.block_until_ready(z)
        t0 = _time.perf_counter_ns()
        outs = sharded(*staged, *zeros)
        jax.block_until_ready(outs)
        return _time.perf_counter_ns() - t0

    def run(per_core_maps):
        outs = run_staged(stage(per_core_maps))
        res = np.asarray(outs[0]).reshape(NCORES, *out_avals[0].shape)
        return res

    _state['run'] = run
    _state['stage'] = stage
    _state['run_staged'] = run_staged
    _state['run_staged_timed'] = run_staged_timed
    _state['nc'] = nc
    _state['in_names'] = in_names
    return run


def make_in_maps(inputs):
    w = prep_weights(inputs)
    x = np.ascontiguousarray(np.asarray(inputs['x'], F32)
                             .reshape(NCORES, BL, N, C))
    gt = np.ascontiguousarray(np.asarray(inputs['group_tokens'], F32)
                              .reshape(NCORES, BL, G_IN, C))
    maps = []
    for c in range(NCORES):
        m = dict(w)
        m['x_l'] = x[c]
        m['gt_l'] = gt[c]
        maps.append(m)
    return maps


def kernel(**inputs):
    run = _get_runner()
    res = run(make_in_maps(inputs))          # [8, BL, 64, 768]
    return np.ascontiguousarray(res.reshape(B, G_OUT, C).astype(np.float32))
